# revision 11
# baseline (speedup 1.0000x reference)
"""Trainium2 Bass kernel for nn_LocallyConnected3 (B=128, C_in=32, C_out=8, S=8192).

  h[b,j,s]  = tanh(x[b,j,s] * sum_i w1[i,j,s])
  out[b,o,s] = tanh(sum_j h[b,j,s] * w2[o,j,s] + bias[o,s])

Sharding: S axis split across 8 cores (1024 positions each).

Per-core layout: SBUF partitions carry (s4, j), s4 in 0..3 (position
sub-block, stride 256) and j in 0..31 (in-channel); positions map to
s_local = s4*256 + u with u in 0..255.  Work is a flat pipeline of 8
chunks of 32 u each; every chunk: DMA x/w1 (f16, packed host-side) ->
vector reduce w1 over i + mul -> scalar tanh -> per 16-u PSUM bank a
bias opener matmul (ones x bias row, N=512) plus 16 packed matmuls
(lhsT = h[(s4,j), b] stationary, rhs = block-diag w2, k=128 contracts
j for 4 positions at once) -> scalar tanh psum->sbuf f16 -> gpsimd
(SWDGE) store.  All engines pipeline chunk-wise; loads ride the SP HW
queue, stores the gpsimd SW queue so neither blocks the other.
"""
import sys

sys.path.insert(0, '/opt/trn_rl_repo')

import numpy as np

import concourse.bass as bass
import concourse.tile as tile
from concourse import mybir
from concourse.bass_utils import run_bass_kernel_spmd

N_CORES = 8
B = 128          # batch
CJ = 32          # C_in
CO = 8           # C_out
S = 8192
SC = S // N_CORES   # 1024 positions per core
NU = SC // 4        # 256 u positions (x4 s4 sub-blocks)
NBANK = 16          # psum-bank work units per core
UB = NU // NBANK    # 16 u per bank
# chunk schedule: small lead-in/lead-out chunks shorten pipeline ramp/drain
CHUNKS = (16, 16, 32, 32, 32, 32, 32, 32, 32)
assert sum(CHUNKS) == NU
NSM = sum(1 for c in CHUNKS if c == 16)   # count of small chunks
NBG = sum(1 for c in CHUNKS if c == 32)   # count of big chunks
F32 = mybir.dt.float32
F16 = mybir.dt.float16


def _patch_tile_drain():
    """core_v3 CTRL instructions accept a single sync-wait; stock
    TileContext packs every final sem wait onto one InstDrain and the pinned
    neuronxcc rejects it.  Spread the waits over single-wait nops."""
    from concourse.tile import ScopedClock, TileContext

    if getattr(TileContext, '_drain_patched', False):
        return

    def _drain_and_barrier_split(self, tick_clock, wait_clock):
        nc = self.nc
        drain_inst = nc.sync.drain()
        wait_clock.add_sem_waits(
            drain_inst.ins, ScopedClock({None: tick_clock.global_clock})
        )
        si = drain_inst.ins.sync_info
        if si is not None and si.on_wait and len(si.on_wait) > 1:
            waits = list(si.on_wait)
            si.on_wait = waits[:1]
            for w in waits[1:]:
                nop = nc.sync.nop(nofuse=True, hint="drain_wait_split")
                nsi = nop.ins.sync_info
                if nsi is None:
                    import bass_rust
                    nop.ins.sync_info = bass_rust.SyncInfo(on_wait=[w], on_update=[])
                else:
                    nsi.on_wait = [w]
        nc.all_engine_barrier()
        assert self.sems is not None
        popped = nc._tile_sem_poison_stack.pop()
        assert popped is self._sem_poison
        nc.clear_and_free_semaphores(list(self.sems.allocated().values()))
        nc.all_engine_barrier()

    TileContext._drain_and_barrier = _drain_and_barrier_split
    TileContext._drain_patched = True


def _build_nc():
    nc = bass.Bass("TRN2")
    # host-packed per core, split small(16u)/big(32u) chunk groups:
    # x: [ch, p=(s4,j), b, ul]; w1: [ch, p, i, ul];
    # w2 block-diag: [ch, p, ul, col=(o*4+s4)]; out: [ch, b, ul, col]
    x_ds = (nc.declare_dram_parameter("xpA", [NSM, 128, B, 16], F16, isOutput=False),
            nc.declare_dram_parameter("xpB", [NBG, 128, B, 32], F16, isOutput=False))
    w1_ds = (nc.declare_dram_parameter("w1A", [NSM, 128, CJ, 16], F16, isOutput=False),
             nc.declare_dram_parameter("w1B", [NBG, 128, CJ, 32], F16, isOutput=False))
    w2_ds = (nc.declare_dram_parameter("w2A", [NSM, 128, 16, 32], F16, isOutput=False),
             nc.declare_dram_parameter("w2B", [NBG, 128, 32, 32], F16, isOutput=False))
    out_ds = (nc.declare_dram_parameter("outA", [NSM, B, 16, 32], F16, isOutput=True),
              nc.declare_dram_parameter("outB", [NBG, B, 32, 32], F16, isOutput=True))
    # host-packed bias: [bank, si, col=(o*4+s4)]
    bias_d = nc.declare_dram_parameter("biasb", [NBANK, UB, 32], F16, isOutput=False)

    with tile.TileContext(nc) as tc:
        with (
            tc.tile_pool(name="xp", bufs=3) as xp,
            tc.tile_pool(name="w1p", bufs=3) as w1p,
            tc.tile_pool(name="w2p", bufs=3) as w2p,
            tc.tile_pool(name="hp", bufs=3) as hp,
            tc.tile_pool(name="bp", bufs=1) as bp,
            tc.tile_pool(name="sp", bufs=3) as sp,
            tc.tile_pool(name="pp", bufs=3, space="PSUM") as pp,
        ):
            bias_t = bp.tile([1, NBANK, UB, 32], F16)
            ones_t = bp.tile([1, 128], F16)
            nc.vector.memset(ones_t[:], 1.0)

            pend = None          # deferred (st tile, out slot) from prev chunk
            bank0 = 0
            idx = [0, 0]         # per-group running chunk index
            for ci, uc in enumerate(CHUNKS):
                g = 0 if uc == 16 else 1
                k = idx[g]
                idx[g] += 1
                nbk = uc // UB
                xt = xp.tile([128, B, uc], F16)
                nc.sync.dma_start(xt[:], x_ds[g][k])
                w1t = w1p.tile([128, CJ, uc], F16)
                nc.sync.dma_start(w1t[:], w1_ds[g][k])
                w2t = w2p.tile([128, uc, 32], F16)
                nc.sync.dma_start(w2t[:], w2_ds[g][k])
                if ci == 0:
                    nc.sync.dma_start(bias_t[0:1], bias_d[:].unsqueeze(0))
                # reduce w1 over i (tree)
                for step in (16, 8, 4, 2, 1):
                    nc.vector.tensor_add(
                        w1t[:, 0:step, :], w1t[:, 0:step, :],
                        w1t[:, step:2 * step, :],
                    )
                # stage 1: h = tanh(x * w1s)
                ht = hp.tile([128, B, uc], F16)
                nc.vector.tensor_mul(
                    ht[:], xt[:],
                    w1t[:, 0:1, :].broadcast_to([128, B, uc]),
                )
                nc.scalar.activation(
                    ht[:], ht[:], mybir.ActivationFunctionType.Tanh
                )
                # flush previous chunk's stage-2 AFTER this chunk's tanh so
                # the ACT engine never head-of-line blocks on the PE
                if pend is not None:
                    stp, slot = pend
                    nc.scalar.activation(
                        stp[:], slot[0][:], mybir.ActivationFunctionType.Tanh
                    )
                    nc.gpsimd.dma_start(slot[1], stp[:])
                # stage 2 matmuls: per 16-u bank a bias opener + 16 packed mms
                ps = pp.tile([128, uc, 32], F32)
                for bk in range(nbk):
                    psl = ps[:, bk * UB:(bk + 1) * UB, :]
                    nc.tensor.matmul(
                        psl,
                        ones_t[:],
                        bias_t[0:1, bank0 + bk],
                        start=True, stop=False,
                    )
                    for si in range(UB):
                        u = bk * UB + si
                        nc.tensor.matmul(
                            ps[:, u, :],
                            ht[:, :, u],             # lhsT [(s4,j), b]
                            w2t[:, u, :],            # rhs  [(s4,j), (o,s4)]
                            start=False, stop=(si == UB - 1),
                            skip_group_check=True,
                        )
                bank0 += nbk
                st = sp.tile([128, uc, 32], F16)
                pend = (st, (ps, out_ds[g][k]))
            # drain the last chunk
            stp, slot = pend
            nc.scalar.activation(
                stp[:], slot[0][:], mybir.ActivationFunctionType.Tanh
            )
            nc.gpsimd.dma_start(slot[1], stp[:])
    _split_multi_waits(nc)
    return nc


def _split_multi_waits(nc):
    """core_v3 CTRL sync accepts one wait per instruction (2 for EventSem).
    Hoist excess waits onto same-engine nofuse nops inserted just before."""
    for fn in nc.m.functions:
        for blk in fn.blocks:
            insts = list(blk.instructions)
            if not any(
                i.sync_info is not None and i.sync_info.on_wait
                and len(i.sync_info.on_wait) > 1
                for i in insts
            ):
                continue
            new = []
            for inst in insts:
                si = inst.sync_info
                cap = 2 if isinstance(inst, mybir.InstEventSemaphore) else 1
                if si is not None and si.on_wait and len(si.on_wait) > cap:
                    waits = list(si.on_wait)
                    si.on_wait = waits[:cap]
                    for k, w in enumerate(waits[cap:]):
                        new.append(mybir.InstNoOp(
                            name=f"{inst.name}-ws{k}",
                            engine=inst.engine,
                            bass_nofuse=True,
                            sync_info=mybir.SyncInfo(on_wait=[w], on_update=[]),
                        ))
                new.append(inst)
            try:
                blk.instructions = new
            except AttributeError:
                blk.instructions[:] = new


def _pack_inputs(x, w1, w2, bias):
    """Shard on S and build the per-core packed f16 side tensors."""
    x = np.asarray(x, np.float32)
    w1 = np.asarray(w1, np.float32)
    w2 = np.asarray(w2, np.float32)
    bias = np.asarray(bias, np.float32)
    bounds = np.cumsum((0,) + CHUNKS)
    in_maps = []
    for c in range(N_CORES):
        sl = slice(c * SC, (c + 1) * SC)
        # x: (b, j, s4, u) -> [(s4,j), b, u]; then slice per chunk on u
        xc = (x[:, :, sl].reshape(B, CJ, 4, NU)
              .transpose(2, 1, 0, 3).reshape(128, B, NU))
        # w1: (i, j, s4, u) -> [(s4,j), i, u]
        w1c = (w1[:, :, sl].reshape(CJ, CJ, 4, NU)
               .transpose(2, 1, 0, 3).reshape(128, CJ, NU))
        # block-diag w2: [(s4,j), u, o*4+s4] = w2[o, j, c*SC + s4*NU + u]
        w2c = w2[:, :, sl].reshape(CO, CJ, 4, NU)        # o j s4 u
        M = np.zeros((4, CJ, NU, 32), np.float32)
        for s4 in range(4):
            M[s4, :, :, s4::4] = w2c[:, :, s4, :].transpose(1, 2, 0)
        w2b = M.reshape(128, NU, 32)
        # bias: [bank, si, o*4+s4] = bias[o, c*SC + s4*NU + bank*UB + si]
        bc = bias[:, sl].reshape(CO, 4, NBANK, UB)       # o s4 bank si
        biasb = np.ascontiguousarray(
            bc.transpose(2, 3, 0, 1).reshape(NBANK, UB, 32)
        ).astype(np.float16)
        m = {"biasb": biasb}
        for g, uc in ((0, 16), (1, 32)):
            us = [bounds[i] for i, s in enumerate(CHUNKS) if s == uc]
            m["xpA" if g == 0 else "xpB"] = np.ascontiguousarray(
                np.stack([xc[:, :, u0:u0 + uc] for u0 in us])).astype(np.float16)
            m["w1A" if g == 0 else "w1B"] = np.ascontiguousarray(
                np.stack([w1c[:, :, u0:u0 + uc] for u0 in us])).astype(np.float16)
            m["w2A" if g == 0 else "w2B"] = np.ascontiguousarray(
                np.stack([w2b[:, u0:u0 + uc, :] for u0 in us])).astype(np.float16)
        in_maps.append(m)
    return in_maps


_CACHED_NC = None


def kernel(x, w1, w2, bias):
    global _CACHED_NC
    _patch_tile_drain()

    if _CACHED_NC is None:
        _CACHED_NC = _build_nc()
    nc = _CACHED_NC

    in_maps = _pack_inputs(x, w1, w2, bias)
    res = run_bass_kernel_spmd(nc, in_maps, list(range(N_CORES)))
    outs = []
    for c in range(N_CORES):
        oa = np.asarray(res.results[c]["outA"])          # [NSM, b, 16, 32]
        ob = np.asarray(res.results[c]["outB"])          # [NBG, b, 32, 32]
        parts, ia, ib = [], 0, 0
        for uc in CHUNKS:
            if uc == 16:
                parts.append(oa[ia]); ia += 1
            else:
                parts.append(ob[ib]); ib += 1
        full = np.concatenate(parts, axis=1)             # [b, NU, 32]
        oc = (full.reshape(B, NU, CO, 4)
              .transpose(0, 2, 3, 1).reshape(B, CO, SC))
        outs.append(oc)
    return np.concatenate(outs, axis=2).astype(np.float32)


if __name__ == "__main__":
    rng = np.random.default_rng(0)
    x = rng.standard_normal((B, CJ, S), dtype=np.float32)
    w1 = rng.standard_normal((CJ, CJ, S), dtype=np.float32)
    w2 = rng.standard_normal((CO, CJ, S), dtype=np.float32)
    bias = rng.standard_normal((CO, S), dtype=np.float32)
    out = kernel(x=x, w1=w1, w2=w2, bias=bias)
    h = np.tanh(x * w1.sum(0, keepdims=True))
    ref = np.tanh(np.einsum('bjs,ojs->bos', h, w2) + bias[None])
    err = np.abs(out - ref).max() / max(np.abs(ref).max(), 1e-9)
    print("self-check rel err:", err)
